# revision 4
# baseline (speedup 1.0000x reference)
"""Trainium2 Bass kernel for the BEMv13 MoE-LoRA module (bf16, v2).

Computation (per token t, full problem):
  base  = x @ W_base.T + b_base
  w     = softmax(x @ W_router + b_router)        # E=2 experts
  out   = base + sum_e w_e * (x @ A_e.T) @ B_e.T * (alpha/rank)

Host-side algebra (exact): with w1 = 1 - w0,
  out = x @ W_eff.T + b_base + w0 * (x @ A_cat.T) @ Bd.T
  W_eff = W_base + scale*B1@A1   (folded on host, free)
  A_cat = [A0; A1]  [16, D],  Bd = scale*[B0, -B1]  [O, 16]
  w0    = sigmoid(x@(wr0-wr1) + (br0-br1))
so the on-chip routing chain is ONE sigmoid + ONE multiply.

Sharding: tokens (batch*seq = 16384) split across 8 NeuronCores; weights
replicated; no cross-core communication.

On-core algorithm (per core, 2048 tokens, all matmul operands bf16):
  - x pre-transposed AND pre-tiled on host: dram row block t holds the 16
    stationary lhsT tiles [k=128, tok=128] of token-tile t.
  - W^T pre-packed per k-slab [128, KT*O] bf16, resident in SBUF; streamed
    as 16 full k-slabs alternating over TWO DMA queues (sync + vector) so
    arrival (~2x one queue) outpaces single-tile consumption; no pair
    phase needed -> uniform 16-tile main loop.
  - PE warmup: a burst of short dummy matmuls on zeroed scratch right
    after the preamble keeps the PE busy through the ~3.4us HAM window
    while the first DMAs land, so real matmuls start at 2.4GHz.
  - h accumulator: pre-block covers k-slabs 13..15 at tile start, in-loop
    h stops at k=12, so the w0 chain (sigmoid -> mul -> DVE 32x32 block
    transposes) finishes with ~3 matmul-groups of slack and the gt
    LDWEIGHTS hides completely under main-matmul streams.
  - gt transpose on the DVE (4x 32x32 stream-transpose blocks) instead of
    the PE: saves PE time and frees a PSUM bank -> psA bufs=6 gives two
    spare acc banks, so tile t+1 never waits on tile t's drains.
  - b_base added on HOST; drains are pure PSUM->SBUF bf16 copies split
    DVE/ACT; per-tile output halves alternate gpsimd/sync store queues.
  - last tile drains+stores per QUARTER on four queues (gpsimd, sync,
    vector, scalar) so the final HBM flush backlog is one quarter.
"""

import numpy as np

P = 128
D = 2048
O = 2048
KT = D // P            # 16 k-slabs
TOK = 2048             # tokens per core
NT = TOK // P          # 16 token tiles
HN = 18                # 16 LoRA cols + 1 router-diff col + 1 pad
ER = 16                # E*R
HSTOP = KT - 4         # last in-loop h slab (12); preblock covers 13,14,15
SCALE = 16.0 / 8.0
NCORES = 8
NWARM = 26             # PE warmup matmuls (~3.5us at cold clock)

_CACHE = {}


def _build():
    import concourse.tile as tile
    from concourse import bacc, mybir

    f32 = mybir.dt.float32
    bf16 = mybir.dt.bfloat16

    nc = bacc.Bacc("TRN2", target_bir_lowering=False, debug=False)

    # xt: row block t = the 16 stationary lhsT tiles of token-tile t,
    # xt[t*P + p, k*P + j] = x[t*P + j, k*P + p]
    xt_d = nc.dram_tensor("xt", [TOK, D], bf16, kind="ExternalInput")
    # wt: wt[p, k*O + o] = W_eff[o, k*P + p]
    wt_d = nc.dram_tensor("wt", [P, KT * O], bf16, kind="ExternalInput")
    aat_d = nc.dram_tensor("aat", [P, KT * HN], bf16, kind="ExternalInput")
    bt_d = nc.dram_tensor("bt", [ER, O], bf16, kind="ExternalInput")
    brd_d = nc.dram_tensor("brd", [1, 1], f32, kind="ExternalInput")
    out_d = nc.dram_tensor("out", [TOK, O], bf16, kind="ExternalOutput")

    with tile.TileContext(nc) as tc:
        with (
            tc.tile_pool(name="res", bufs=1) as res,
            tc.tile_pool(name="obuf", bufs=2) as obuf,
            tc.tile_pool(name="xpool", bufs=4) as xpool,
            tc.tile_pool(name="small", bufs=2) as small,
            tc.tile_pool(name="psA", bufs=6, space="PSUM") as psA,
            tc.tile_pool(name="psH", bufs=2, space="PSUM") as psH,
        ):
            # --- PE warmup: zeroed scratch, short dummy matmuls.
            wsrc = res.tile([P, 256], bf16, tag="wsrc")
            nc.vector.memset(wsrc[:], 0.0)
            psW = psA.tile([P, 512], f32, tag="acc", name="warm")
            for i in range(NWARM):
                nc.tensor.matmul(psW[:, 0:P], wsrc[:, 0:P], wsrc[:, P:2 * P],
                                 start=True, stop=True)

            # --- even W^T k-slabs on the SP queue (scalar takes the odd
            # ones after the x0/x1 chunks; only SP/Act/Pool can DMA).
            wt_b = res.tile([P, KT * O], bf16, tag="wt_b")

            def load_w(k):
                q = nc.sync if k % 2 == 0 else nc.scalar
                q.dma_start(wt_b[:, k * O:(k + 1) * O],
                            wt_d[:, k * O:(k + 1) * O])

            for k in range(0, KT, 2):
                load_w(k)

            # x tok-tile loads: x0/x1 up front on the ACT HWDGE queue
            # (before the odd W slabs), x2+ on the Pool queue.
            x_tiles = [None] * NT

            def load_x(t, chunks=1, q=None):
                q = q or nc.gpsimd
                x_tiles[t] = xpool.tile([P, D], bf16, tag="x", name=f"x_{t}")
                cw = D // chunks
                for cc in range(chunks):
                    q.dma_start(
                        x_tiles[t][:, cc * cw:(cc + 1) * cw],
                        xt_d[t * P:(t + 1) * P, cc * cw:(cc + 1) * cw])

            # small constants first on the Pool queue (aat: needed by h k0).
            aat_b = res.tile([P, KT * HN], bf16, tag="aat_b")
            nc.gpsimd.dma_start(aat_b[:], aat_d[:])

            load_x(0, chunks=4, q=nc.scalar)
            load_x(1, chunks=2, q=nc.scalar)
            for k in range(1, KT, 2):
                load_w(k)
            load_x(2)
            load_x(3)

            bt_b = res.tile([ER, O], bf16, tag="bt_b")
            nc.gpsimd.dma_start(bt_b[:], bt_d[:])
            # router bias diff (b_r0 - b_r1), partition-broadcast
            brd128 = res.tile([P, 1], f32, tag="brd128")
            nc.gpsimd.dma_start(brd128[:], brd_d[:].broadcast_to((P, 1)))

            def lhs(t, k):
                return x_tiles[t][:, k * P:(k + 1) * P]

            # routing weight + scaled-H transpose; returns gt [32, P] bf16
            # (partitions 16..31 are zero padding for the 32x32 transpose).
            def make_gt(t, h):
                w0s = small.tile([P, 1], f32, tag="w0s", name=f"w0s_{t}")
                nc.scalar.activation(w0s[:], h[:, ER:ER + 1],
                                     mybir.ActivationFunctionType.Sigmoid,
                                     bias=brd128[:, 0:1], scale=1.0)
                g = small.tile([P, 32], bf16, tag="g", name=f"g_{t}")
                nc.vector.memset(g[:, ER:32], 0.0)
                nc.vector.tensor_scalar_mul(g[:, 0:ER], h[:, 0:ER], w0s[:])
                gt = small.tile([32, P], bf16, tag="gt", name=f"gt_{t}")
                for r in range(4):
                    nc.vector.transpose(gt[0:32, r * 32:(r + 1) * 32],
                                        g[r * 32:(r + 1) * 32, 0:32])
                return gt

            # h pre-block: open the h accumulation with k-slabs 13..15 so the
            # in-loop h stops at k=12, three matmul groups before tile end.
            def h_preblock(t, h):
                for k in range(HSTOP + 1, KT):
                    nc.tensor.matmul(h[:], lhs(t, k),
                                     aat_b[:, k * HN:(k + 1) * HN],
                                     start=(k == HSTOP + 1), stop=False)

            # drain acc j of tile t as a pure copy (bias added on host);
            # even j on DVE, odd j on ACT.
            def drain(t, acc, j, cols=None):
                dst = out_tiles[t][:, j * 512:(j + 1) * 512]
                if cols is not None:
                    dst = out_tiles[t][:, cols[0]:cols[1]]
                    acc = acc[:, cols[0] - j * 512:cols[1] - j * 512]
                if j % 2 == 0:
                    nc.vector.tensor_copy(dst, acc[:])
                else:
                    nc.scalar.copy(dst, acc[:])

            out_tiles = [None] * NT

            # =========== uniform main loop: tiles 0..15 ====================
            for t in range(NT):
                out_tiles[t] = obuf.tile([P, O], bf16, tag="obuf", name=f"out_{t}")
                accs = [psA.tile([P, 512], f32, tag="acc", name=f"acc_{t}_{j}")
                        for j in range(4)]
                h = psH.tile([P, HN], f32, tag="h", name=f"h_{t}")
                h_preblock(t, h)
                gt = None
                for k in range(KT):
                    if k <= HSTOP:
                        nc.tensor.matmul(h[:], lhs(t, k),
                                         aat_b[:, k * HN:(k + 1) * HN],
                                         start=False, stop=(k == HSTOP))
                    for j in range(4):
                        nc.tensor.matmul(
                            accs[j][:], lhs(t, k),
                            wt_b[:, k * O + j * 512:k * O + (j + 1) * 512],
                            start=(k == 0), stop=False)
                    if k == HSTOP:
                        gt = make_gt(t, h)
                    if k == 8 and t + 4 < NT:
                        load_x(t + 4)
                for j in range(4):
                    nc.tensor.matmul(accs[j][:], gt[0:ER, :],
                                     bt_b[:, j * 512:(j + 1) * 512],
                                     start=False, stop=True)
                if t == NT - 1:
                    # final tile: per-quarter drains + stores on 3 queues so
                    # the end-of-kernel flush backlog is small.
                    for j, q in zip(range(4),
                                    (nc.gpsimd, nc.sync, nc.scalar, nc.sync)):
                        drain(t, accs[j], j)
                        q.dma_start(
                            out_d[t * P:(t + 1) * P, j * 512:(j + 1) * 512],
                            out_tiles[t][:, j * 512:(j + 1) * 512])
                else:
                    sq = nc.sync if t % 2 == 0 else nc.scalar
                    drain(t, accs[0], 0)
                    drain(t, accs[1], 1)
                    sq.dma_start(out_d[t * P:(t + 1) * P, 0:1024],
                                 out_tiles[t][:, 0:1024])
                    drain(t, accs[2], 2)
                    drain(t, accs[3], 3)
                    sq.dma_start(out_d[t * P:(t + 1) * P, 1024:2048],
                                 out_tiles[t][:, 1024:2048])

    nc.compile()
    return nc


def _prep_host(x, W_base, b_base, A, B, W_router, b_router):
    """Host-side layout prep + sharding. Returns per-core input maps."""
    import ml_dtypes
    bf16 = ml_dtypes.bfloat16

    A = np.asarray(A, dtype=np.float32)
    B = np.asarray(B, dtype=np.float32)
    wr = np.asarray(W_router, dtype=np.float32)

    x_flat = np.ascontiguousarray(x, dtype=np.float32).reshape(-1, D)
    # xt[t*P + p, k*P + j] = x[t*P + j, k*P + p], per core
    NTOT = x_flat.shape[0] // P
    xt_all = np.ascontiguousarray(
        x_flat.reshape(NTOT, P, KT, P).transpose(0, 3, 2, 1)
    ).reshape(NTOT * P, D).astype(bf16)

    # W_eff = W_base + scale*B1@A1, folded on host
    w_eff = np.asarray(W_base, dtype=np.float32) + SCALE * (B[1] @ A[1])
    wt = w_eff.T                                                    # [D, O]
    wt_p = np.ascontiguousarray(
        wt.reshape(KT, P, O).transpose(1, 0, 2).reshape(P, KT * O)
    ).astype(bf16)

    a_cat = A.reshape(ER, D)                                        # [16, D]
    aat = np.zeros((D, HN), dtype=np.float32)
    aat[:, :ER] = a_cat.T
    aat[:, ER] = wr[:, 0] - wr[:, 1]
    aat_p = np.ascontiguousarray(
        aat.reshape(KT, P, HN).transpose(1, 0, 2).reshape(P, KT * HN)
    ).astype(bf16)

    b_d = np.concatenate([B[0], -B[1]], axis=1)                     # [O, 16]
    bt = np.ascontiguousarray(b_d.T * SCALE).astype(bf16)           # [16, O]
    dlb = np.float32(b_router[0]) - np.float32(b_router[1])
    brd = np.array([[dlb]], dtype=np.float32)

    in_maps = []
    for c in range(NCORES):
        in_maps.append({
            "xt": xt_all[c * TOK:(c + 1) * TOK],
            "wt": wt_p,
            "aat": aat_p,
            "bt": bt,
            "brd": brd,
        })
    return in_maps


def kernel(x, W_base, b_base, A, B, W_router, b_router):
    from concourse import bass_utils

    if "nc" not in _CACHE:
        _CACHE["nc"] = _build()
    nc = _CACHE["nc"]

    in_maps = _prep_host(x, W_base, b_base, A, B, W_router, b_router)
    res = None
    for attempt in range(3):
        try:
            res = bass_utils.run_bass_kernel_spmd(
                nc, in_maps, core_ids=list(range(NCORES)))
            break
        except Exception:
            # rare transient NRT_EXEC_UNIT_UNRECOVERABLE observed once;
            # the same NEFF runs fine on retry
            if attempt == 2:
                raise
    out = np.concatenate([res.results[c]["out"] for c in range(NCORES)], axis=0)
    out = out.astype(np.float32) + np.asarray(b_base, dtype=np.float32)
    return out.reshape(np.asarray(x).shape[0], -1, O)


# revision 6
# speedup vs baseline: 1.0291x; 1.0291x over previous
"""Trainium2 Bass kernel for the BEMv13 MoE-LoRA module (bf16, v2).

Computation (per token t, full problem):
  base  = x @ W_base.T + b_base
  w     = softmax(x @ W_router + b_router)        # E=2 experts
  out   = base + sum_e w_e * (x @ A_e.T) @ B_e.T * (alpha/rank)

Host-side algebra (exact): with w1 = 1 - w0,
  out = x @ W_eff.T + b_base + w0 * (x @ A_cat.T) @ Bd.T
  W_eff = W_base + scale*B1@A1   (folded on host, free)
  A_cat = [A0; A1]  [16, D],  Bd = scale*[B0, -B1]  [O, 16]
  w0    = sigmoid(x@(wr0-wr1) + (br0-br1))
so the on-chip routing chain is ONE sigmoid + ONE multiply.

Sharding: tokens (batch*seq = 16384) split across 8 NeuronCores; weights
replicated; no cross-core communication.

On-core algorithm (per core, 2048 tokens, all matmul operands bf16):
  - x pre-transposed AND pre-tiled on host: dram row block t holds the 16
    stationary lhsT tiles [k=128, tok=128] of token-tile t.
  - W^T pre-packed per k-slab [128, KT*O] bf16, resident in SBUF; streamed
    as 16 full k-slabs alternating over TWO DMA queues (sync + vector) so
    arrival (~2x one queue) outpaces single-tile consumption; no pair
    phase needed -> uniform 16-tile main loop.
  - PE warmup: a burst of short dummy matmuls on zeroed scratch right
    after the preamble keeps the PE busy through the ~3.4us HAM window
    while the first DMAs land, so real matmuls start at 2.4GHz.
  - h accumulator: pre-block covers k-slabs 13..15 at tile start, in-loop
    h stops at k=12, so the w0 chain (sigmoid -> mul -> DVE 32x32 block
    transposes) finishes with ~3 matmul-groups of slack and the gt
    LDWEIGHTS hides completely under main-matmul streams.
  - gt transpose on the DVE (4x 32x32 stream-transpose blocks) instead of
    the PE: saves PE time and frees a PSUM bank -> psA bufs=6 gives two
    spare acc banks, so tile t+1 never waits on tile t's drains.
  - b_base added on HOST; drains are pure PSUM->SBUF bf16 copies split
    DVE/ACT; per-tile output halves alternate gpsimd/sync store queues.
  - last tile drains+stores per QUARTER on four queues (gpsimd, sync,
    vector, scalar) so the final HBM flush backlog is one quarter.
"""

import numpy as np

P = 128
D = 2048
O = 2048
KT = D // P            # 16 k-slabs
TOK = 2048             # tokens per core
NT = TOK // P          # 16 token tiles
HN = 18                # 16 LoRA cols + 1 router-diff col + 1 pad
ER = 16                # E*R
HSTOP = KT - 4         # last in-loop h slab (12); preblock covers 13,14,15
SCALE = 16.0 / 8.0
NCORES = 8
NWARM = 26             # PE warmup matmuls (~3.5us at cold clock)

_CACHE = {}


def _build():
    import concourse.tile as tile
    from concourse import bacc, mybir

    f32 = mybir.dt.float32
    bf16 = mybir.dt.bfloat16

    nc = bacc.Bacc("TRN2", target_bir_lowering=False, debug=False)

    # xt: row block t = the 16 stationary lhsT tiles of token-tile t,
    # xt[t*P + p, k*P + j] = x[t*P + j, k*P + p]
    xt_d = nc.dram_tensor("xt", [TOK, D], bf16, kind="ExternalInput")
    # wt: wt[p, k*O + o] = W_eff[o, k*P + p]
    wt_d = nc.dram_tensor("wt", [P, KT * O], bf16, kind="ExternalInput")
    aat_d = nc.dram_tensor("aat", [P, KT * HN], bf16, kind="ExternalInput")
    bt_d = nc.dram_tensor("bt", [ER, O], bf16, kind="ExternalInput")
    brd_d = nc.dram_tensor("brd", [1, 1], f32, kind="ExternalInput")
    out_d = nc.dram_tensor("out", [TOK, O], bf16, kind="ExternalOutput")

    with tile.TileContext(nc) as tc:
        with (
            tc.tile_pool(name="res", bufs=1) as res,
            tc.tile_pool(name="obuf", bufs=2) as obuf,
            tc.tile_pool(name="xpool", bufs=4) as xpool,
            tc.tile_pool(name="small", bufs=2) as small,
            tc.tile_pool(name="psA", bufs=6, space="PSUM") as psA,
            tc.tile_pool(name="psH", bufs=2, space="PSUM") as psH,
        ):
            # --- PE warmup: zeroed scratch, short dummy matmuls.
            wsrc = res.tile([P, 256], bf16, tag="wsrc")
            nc.vector.memset(wsrc[:], 0.0)
            psW = psA.tile([P, 512], f32, tag="acc", name="warm")
            for i in range(NWARM):
                nc.tensor.matmul(psW[:, 0:P], wsrc[:, 0:P], wsrc[:, P:2 * P],
                                 start=True, stop=True)

            # --- W^T stream: SP queue ONLY, half-slabs in (half, k) order so
            # the startup pair phase consumes them in arrival order. The
            # first ~10 DMA ring slots are precious (8 shared rings, ~3us
            # completion lag each): only W + x0/x1 may occupy them.
            HO = O // 2
            wt_b = res.tile([P, KT * O], bf16, tag="wt_b")
            for hh in range(2):
                for k in range(KT):
                    nc.sync.dma_start(
                        wt_b[:, k * O + hh * HO:k * O + (hh + 1) * HO],
                        wt_d[:, k * O + hh * HO:k * O + (hh + 1) * HO])

            # x tok-tile loads + small constants on the ACT HWDGE queue.
            x_tiles = [None] * NT

            def load_x(t, chunks=1):
                x_tiles[t] = xpool.tile([P, D], bf16, tag="x", name=f"x_{t}")
                cw = D // chunks
                for cc in range(chunks):
                    nc.scalar.dma_start(
                        x_tiles[t][:, cc * cw:(cc + 1) * cw],
                        xt_d[t * P:(t + 1) * P, cc * cw:(cc + 1) * cw])

            load_x(0, chunks=4)
            load_x(1, chunks=2)

            aat_b = res.tile([P, KT * HN], bf16, tag="aat_b")
            nc.scalar.dma_start(aat_b[:], aat_d[:])
            bt_b = res.tile([ER, O], bf16, tag="bt_b")
            nc.scalar.dma_start(bt_b[:], bt_d[:])
            # router bias diff (b_r0 - b_r1), partition-broadcast
            brd128 = res.tile([P, 1], f32, tag="brd128")
            nc.gpsimd.dma_start(brd128[:], brd_d[:].broadcast_to((P, 1)))

            def lhs(t, k):
                return x_tiles[t][:, k * P:(k + 1) * P]

            # routing weight + scaled-H transpose; returns gt [32, P] bf16
            # (partitions 16..31 are zero padding for the 32x32 transpose).
            def make_gt(t, h):
                w0s = small.tile([P, 1], f32, tag="w0s", name=f"w0s_{t}")
                nc.scalar.activation(w0s[:], h[:, ER:ER + 1],
                                     mybir.ActivationFunctionType.Sigmoid,
                                     bias=brd128[:, 0:1], scale=1.0)
                g = small.tile([P, 32], bf16, tag="g", name=f"g_{t}")
                nc.vector.memset(g[:, ER:32], 0.0)
                nc.vector.tensor_scalar_mul(g[:, 0:ER], h[:, 0:ER], w0s[:])
                gt = small.tile([32, P], bf16, tag="gt", name=f"gt_{t}")
                for r in range(4):
                    nc.vector.transpose(gt[0:32, r * 32:(r + 1) * 32],
                                        g[r * 32:(r + 1) * 32, 0:32])
                return gt

            # h pre-block: open the h accumulation with k-slabs 13..15 so the
            # in-loop h stops at k=12, three matmul groups before tile end.
            def h_preblock(t, h):
                for k in range(HSTOP + 1, KT):
                    nc.tensor.matmul(h[:], lhs(t, k),
                                     aat_b[:, k * HN:(k + 1) * HN],
                                     start=(k == HSTOP + 1), stop=False)

            # drain acc j of tile t as a pure copy (bias added on host);
            # even j on DVE, odd j on ACT.
            def drain(t, acc, j, cols=None):
                dst = out_tiles[t][:, j * 512:(j + 1) * 512]
                if cols is not None:
                    dst = out_tiles[t][:, cols[0]:cols[1]]
                    acc = acc[:, cols[0] - j * 512:cols[1] - j * 512]
                if j % 2 == 0:
                    nc.vector.tensor_copy(dst, acc[:])
                else:
                    nc.scalar.copy(dst, acc[:])

            def store(t, half):
                nc.gpsimd.dma_start(
                    out_d[t * P:(t + 1) * P, half * HO:(half + 1) * HO],
                    out_tiles[t][:, half * HO:(half + 1) * HO])

            out_tiles = [None] * NT

            # =========== startup: tiles 0,1 as a pair, half-O per pass =====
            # (consumption 4x512 cycles per W half-slab tracks the ~650ns
            # single-queue W arrival cadence; single-tile would stall)
            hps = [psH.tile([P, HN], f32, tag="h", name=f"h_{t}")
                   for t in range(2)]
            gts = [None, None]
            for t in range(2):
                out_tiles[t] = obuf.tile([P, O], bf16, tag="obuf", name=f"out_{t}")

            for hh in range(2):
                accs = [[psA.tile([P, 512], f32, tag="acc", name=f"acc_{t}_{hh}_{j}")
                         for j in range(2)] for t in range(2)]
                if hh == 0:
                    for t in range(2):
                        h_preblock(t, hps[t])
                for k in range(KT):
                    for t in range(2):
                        if hh == 0 and k <= HSTOP:
                            nc.tensor.matmul(hps[t][:], lhs(t, k),
                                             aat_b[:, k * HN:(k + 1) * HN],
                                             start=False, stop=(k == HSTOP))
                        for j in range(2):
                            nc.tensor.matmul(
                                accs[t][j][:], lhs(t, k),
                                wt_b[:, k * O + hh * HO + j * 512:
                                     k * O + hh * HO + (j + 1) * 512],
                                start=(k == 0), stop=False)
                    if hh == 0 and k == HSTOP:
                        gts[0] = make_gt(0, hps[0])
                        gts[1] = make_gt(1, hps[1])
                for t in range(2):
                    for j in range(2):
                        nc.tensor.matmul(accs[t][j][:], gts[t][0:ER, :],
                                         bt_b[:, hh * HO + j * 512:
                                              hh * HO + (j + 1) * 512],
                                         start=False, stop=True)
                for t in range(2):
                    for j in range(2):
                        drain(t, accs[t][j], 2 * hh + j)
                for t in range(2):
                    store(t, hh)

            load_x(2)
            load_x(3)

            # =========== main loop: tiles 2..15, one tile at a time ========
            for t in range(2, NT):
                out_tiles[t] = obuf.tile([P, O], bf16, tag="obuf", name=f"out_{t}")
                accs = [psA.tile([P, 512], f32, tag="acc", name=f"acc_{t}_{j}")
                        for j in range(4)]
                h = psH.tile([P, HN], f32, tag="h", name=f"h_{t}")
                h_preblock(t, h)
                gt = None
                for k in range(KT):
                    if k <= HSTOP:
                        nc.tensor.matmul(h[:], lhs(t, k),
                                         aat_b[:, k * HN:(k + 1) * HN],
                                         start=False, stop=(k == HSTOP))
                    for j in range(4):
                        nc.tensor.matmul(
                            accs[j][:], lhs(t, k),
                            wt_b[:, k * O + j * 512:k * O + (j + 1) * 512],
                            start=(k == 0), stop=False)
                    if k == HSTOP:
                        gt = make_gt(t, h)
                    if k == 8 and t + 2 < NT:
                        load_x(t + 2)
                for j in range(4):
                    nc.tensor.matmul(accs[j][:], gt[0:ER, :],
                                     bt_b[:, j * 512:(j + 1) * 512],
                                     start=False, stop=True)
                if t == NT - 1:
                    # final tile: per-quarter drains + stores on 3 queues so
                    # the end-of-kernel flush backlog is small.
                    for j, q in zip(range(4),
                                    (nc.gpsimd, nc.sync, nc.scalar, nc.sync)):
                        drain(t, accs[j], j)
                        q.dma_start(
                            out_d[t * P:(t + 1) * P, j * 512:(j + 1) * 512],
                            out_tiles[t][:, j * 512:(j + 1) * 512])
                else:
                    drain(t, accs[0], 0)
                    drain(t, accs[1], 1)
                    store(t, 0)
                    drain(t, accs[2], 2)
                    drain(t, accs[3], 3)
                    store(t, 1)

    nc.compile()
    return nc


def _prep_host(x, W_base, b_base, A, B, W_router, b_router):
    """Host-side layout prep + sharding. Returns per-core input maps."""
    import ml_dtypes
    bf16 = ml_dtypes.bfloat16

    A = np.asarray(A, dtype=np.float32)
    B = np.asarray(B, dtype=np.float32)
    wr = np.asarray(W_router, dtype=np.float32)

    x_flat = np.ascontiguousarray(x, dtype=np.float32).reshape(-1, D)
    # xt[t*P + p, k*P + j] = x[t*P + j, k*P + p], per core
    NTOT = x_flat.shape[0] // P
    xt_all = np.ascontiguousarray(
        x_flat.reshape(NTOT, P, KT, P).transpose(0, 3, 2, 1)
    ).reshape(NTOT * P, D).astype(bf16)

    # W_eff = W_base + scale*B1@A1, folded on host
    w_eff = np.asarray(W_base, dtype=np.float32) + SCALE * (B[1] @ A[1])
    wt = w_eff.T                                                    # [D, O]
    wt_p = np.ascontiguousarray(
        wt.reshape(KT, P, O).transpose(1, 0, 2).reshape(P, KT * O)
    ).astype(bf16)

    a_cat = A.reshape(ER, D)                                        # [16, D]
    aat = np.zeros((D, HN), dtype=np.float32)
    aat[:, :ER] = a_cat.T
    aat[:, ER] = wr[:, 0] - wr[:, 1]
    aat_p = np.ascontiguousarray(
        aat.reshape(KT, P, HN).transpose(1, 0, 2).reshape(P, KT * HN)
    ).astype(bf16)

    b_d = np.concatenate([B[0], -B[1]], axis=1)                     # [O, 16]
    bt = np.ascontiguousarray(b_d.T * SCALE).astype(bf16)           # [16, O]
    dlb = np.float32(b_router[0]) - np.float32(b_router[1])
    brd = np.array([[dlb]], dtype=np.float32)

    in_maps = []
    for c in range(NCORES):
        in_maps.append({
            "xt": xt_all[c * TOK:(c + 1) * TOK],
            "wt": wt_p,
            "aat": aat_p,
            "bt": bt,
            "brd": brd,
        })
    return in_maps


def kernel(x, W_base, b_base, A, B, W_router, b_router):
    from concourse import bass_utils

    if "nc" not in _CACHE:
        _CACHE["nc"] = _build()
    nc = _CACHE["nc"]

    in_maps = _prep_host(x, W_base, b_base, A, B, W_router, b_router)
    res = None
    for attempt in range(3):
        try:
            res = bass_utils.run_bass_kernel_spmd(
                nc, in_maps, core_ids=list(range(NCORES)))
            break
        except Exception:
            # rare transient NRT_EXEC_UNIT_UNRECOVERABLE observed once;
            # the same NEFF runs fine on retry
            if attempt == 2:
                raise
    out = np.concatenate([res.results[c]["out"] for c in range(NCORES)], axis=0)
    out = out.astype(np.float32) + np.asarray(b_base, dtype=np.float32)
    return out.reshape(np.asarray(x).shape[0], -1, O)


# revision 12
# speedup vs baseline: 1.0457x; 1.0161x over previous
"""Trainium2 Bass kernel for the BEMv13 MoE-LoRA module (bf16, v2).

Computation (per token t, full problem):
  base  = x @ W_base.T + b_base
  w     = softmax(x @ W_router + b_router)        # E=2 experts
  out   = base + sum_e w_e * (x @ A_e.T) @ B_e.T * (alpha/rank)

Host-side algebra (exact): with w1 = 1 - w0,
  out = x @ W_eff.T + b_base + w0 * (x @ A_cat.T) @ Bd.T
  W_eff = W_base + scale*B1@A1   (folded on host, free)
  A_cat = [A0; A1]  [16, D],  Bd = scale*[B0, -B1]  [O, 16]
  w0    = sigmoid(x@(wr0-wr1) + (br0-br1))
so the on-chip routing chain is ONE sigmoid + ONE multiply.

Sharding: tokens (batch*seq = 16384) split across 8 NeuronCores; weights
replicated; no cross-core communication.

On-core algorithm (per core, 2048 tokens, all matmul operands bf16):
  - x pre-transposed AND pre-tiled on host: dram row block t holds the 16
    stationary lhsT tiles [k=128, tok=128] of token-tile t.
  - W^T pre-packed per k-slab [128, KT*O] bf16, resident in SBUF; streamed
    as 16 full k-slabs alternating over TWO DMA queues (sync + vector) so
    arrival (~2x one queue) outpaces single-tile consumption; no pair
    phase needed -> uniform 16-tile main loop.
  - PE warmup: a burst of short dummy matmuls on zeroed scratch right
    after the preamble keeps the PE busy through the ~3.4us HAM window
    while the first DMAs land, so real matmuls start at 2.4GHz.
  - h accumulator: pre-block covers k-slabs 13..15 at tile start, in-loop
    h stops at k=12, so the w0 chain (sigmoid -> mul -> DVE 32x32 block
    transposes) finishes with ~3 matmul-groups of slack and the gt
    LDWEIGHTS hides completely under main-matmul streams.
  - gt transpose on the DVE (4x 32x32 stream-transpose blocks) instead of
    the PE: saves PE time and frees a PSUM bank -> psA bufs=6 gives two
    spare acc banks, so tile t+1 never waits on tile t's drains.
  - b_base added on HOST; drains are pure PSUM->SBUF bf16 copies split
    DVE/ACT; per-tile output halves alternate gpsimd/sync store queues.
  - last tile drains+stores per QUARTER on four queues (gpsimd, sync,
    vector, scalar) so the final HBM flush backlog is one quarter.
"""

import numpy as np

P = 128
D = 2048
O = 2048
KT = D // P            # 16 k-slabs
TOK = 2048             # tokens per core
NT = TOK // P          # 16 token tiles
HN = 18                # 16 LoRA cols + 1 router-diff col + 1 pad
ER = 16                # E*R
HSTOP = KT - 4         # last in-loop h slab (12); preblock covers 13,14,15
SCALE = 16.0 / 8.0
NCORES = 8

_CACHE = {}


def _build():
    import concourse.tile as tile
    from concourse import bacc, mybir

    f32 = mybir.dt.float32
    bf16 = mybir.dt.bfloat16

    nc = bacc.Bacc("TRN2", target_bir_lowering=False, debug=False)

    # xt: row block t = the 16 stationary lhsT tiles of token-tile t,
    # xt[t*P + p, k*P + j] = x[t*P + j, k*P + p]
    xt_d = nc.dram_tensor("xt", [TOK, D], bf16, kind="ExternalInput")
    # wt: wt[p, k*O + o] = W_eff[o, k*P + p]
    wt_d = nc.dram_tensor("wt", [P, KT * O], bf16, kind="ExternalInput")
    aat_d = nc.dram_tensor("aat", [P, KT * HN], bf16, kind="ExternalInput")
    # bt zero-padded to 128 contraction rows: keeping the G-update matmuls
    # at the standard 128x128 stationary tile size avoids the PE
    # tile-config switch penalty that a 16-row stationary incurs.
    bt_d = nc.dram_tensor("bt", [P, O], bf16, kind="ExternalInput")
    brd_d = nc.dram_tensor("brd", [1, 1], f32, kind="ExternalInput")
    out_d = nc.dram_tensor("out", [TOK, O], bf16, kind="ExternalOutput")

    with tile.TileContext(nc) as tc:
        with (
            tc.tile_pool(name="res", bufs=1) as res,
            tc.tile_pool(name="obuf", bufs=2) as obuf,
            tc.tile_pool(name="xpool", bufs=4) as xpool,
            tc.tile_pool(name="small", bufs=2) as small,
            tc.tile_pool(name="psA", bufs=6, space="PSUM") as psA,
            tc.tile_pool(name="psH", bufs=2, space="PSUM") as psH,
        ):
            # --- W^T stream: SP queue ONLY, half-slabs in (half, k) order so
            # the startup pair phase consumes them in arrival order. The
            # first ~10 DMA ring slots are precious (8 shared rings, ~3us
            # completion lag each): only W + x0/x1 may occupy them.
            HO = O // 2
            wt_b = res.tile([P, KT * O], bf16, tag="wt_b")
            for hh in range(2):
                for k in range(KT):
                    nc.sync.dma_start(
                        wt_b[:, k * O + hh * HO:k * O + (hh + 1) * HO],
                        wt_d[:, k * O + hh * HO:k * O + (hh + 1) * HO])

            # x tok-tile loads + small constants on the ACT HWDGE queue.
            x_tiles = [None] * NT

            def load_x(t, chunks=1):
                x_tiles[t] = xpool.tile([P, D], bf16, tag="x", name=f"x_{t}")
                cw = D // chunks
                for cc in range(chunks):
                    nc.scalar.dma_start(
                        x_tiles[t][:, cc * cw:(cc + 1) * cw],
                        xt_d[t * P:(t + 1) * P, cc * cw:(cc + 1) * cw])

            # chunk order: x0 cols 0:1024, then x1 whole (the pair phase
            # zippers t0/t1 per k-slab, so x1 is needed almost immediately),
            # then x0's tail, then the small constants.
            x_tiles[0] = xpool.tile([P, D], bf16, tag="x", name="x_0")
            x_tiles[1] = xpool.tile([P, D], bf16, tag="x", name="x_1")
            for cc in range(2):
                nc.scalar.dma_start(x_tiles[0][:, cc * 512:(cc + 1) * 512],
                                    xt_d[0:P, cc * 512:(cc + 1) * 512])
            for cc in range(2):
                nc.scalar.dma_start(x_tiles[1][:, cc * 1024:(cc + 1) * 1024],
                                    xt_d[P:2 * P, cc * 1024:(cc + 1) * 1024])
            for cc in range(2, 4):
                nc.scalar.dma_start(x_tiles[0][:, cc * 512:(cc + 1) * 512],
                                    xt_d[0:P, cc * 512:(cc + 1) * 512])

            aat_b = res.tile([P, KT * HN], bf16, tag="aat_b")
            nc.scalar.dma_start(aat_b[:], aat_d[:])
            bt_b = res.tile([P, O], bf16, tag="bt_b")
            nc.scalar.dma_start(bt_b[:], bt_d[:])
            # router bias diff (b_r0 - b_r1), partition-broadcast
            brd128 = res.tile([P, 1], f32, tag="brd128")
            nc.gpsimd.dma_start(brd128[:], brd_d[:].broadcast_to((P, 1)))

            def lhs(t, k):
                return x_tiles[t][:, k * P:(k + 1) * P]

            # routing weight + scaled-H transpose; returns gt [P, P] bf16
            # (partitions 16..127 are zero so the G matmul keeps the
            # standard 128-row stationary tile size).
            def make_gt(t, h):
                w0s = small.tile([P, 1], f32, tag="w0s", name=f"w0s_{t}")
                nc.scalar.activation(w0s[:], h[:, ER:ER + 1],
                                     mybir.ActivationFunctionType.Sigmoid,
                                     bias=brd128[:, 0:1], scale=1.0)
                g = small.tile([P, 32], bf16, tag="g", name=f"g_{t}")
                nc.vector.memset(g[:, ER:32], 0.0)
                nc.vector.tensor_scalar_mul(g[:, 0:ER], h[:, 0:ER], w0s[:])
                gt = small.tile([P, P], bf16, tag="gt", name=f"gt_{t}")
                nc.vector.memset(gt[32:64, :], 0.0)
                nc.vector.memset(gt[64:P, :], 0.0)
                for r in range(4):
                    nc.vector.transpose(gt[0:32, r * 32:(r + 1) * 32],
                                        g[r * 32:(r + 1) * 32, 0:32])
                return gt

            # h pre-block: open the h accumulation with k-slabs 13..15 so the
            # in-loop h stops at k=12, three matmul groups before tile end.
            def h_preblock(t, h):
                for k in range(HSTOP + 1, KT):
                    nc.tensor.matmul(h[:], lhs(t, k),
                                     aat_b[:, k * HN:(k + 1) * HN],
                                     start=(k == HSTOP + 1), stop=False)

            # drain acc j of tile t as a pure copy (bias added on host);
            # even j on DVE, odd j on ACT.
            def drain(t, acc, j, cols=None):
                dst = out_tiles[t][:, j * 512:(j + 1) * 512]
                if cols is not None:
                    dst = out_tiles[t][:, cols[0]:cols[1]]
                    acc = acc[:, cols[0] - j * 512:cols[1] - j * 512]
                if j % 2 == 0:
                    nc.vector.tensor_copy(dst, acc[:])
                else:
                    nc.scalar.copy(dst, acc[:])

            def store(t, half):
                nc.gpsimd.dma_start(
                    out_d[t * P:(t + 1) * P, half * HO:(half + 1) * HO],
                    out_tiles[t][:, half * HO:(half + 1) * HO])

            out_tiles = [None] * NT

            # =========== startup: tiles 0,1 as a pair, half-O per pass =====
            # (consumption 4x512 cycles per W half-slab tracks the ~650ns
            # single-queue W arrival cadence; single-tile would stall)
            hps = [psH.tile([P, HN], f32, tag="h", name=f"h_{t}")
                   for t in range(2)]
            gts = [None, None]
            for t in range(2):
                out_tiles[t] = obuf.tile([P, O], bf16, tag="obuf", name=f"out_{t}")

            for hh in range(2):
                accs = [[psA.tile([P, 512], f32, tag="acc", name=f"acc_{t}_{hh}_{j}")
                         for j in range(2)] for t in range(2)]
                if hh == 0:
                    for t in range(2):
                        h_preblock(t, hps[t])
                for k in range(KT):
                    for t in range(2):
                        if hh == 0 and k <= HSTOP:
                            nc.tensor.matmul(hps[t][:], lhs(t, k),
                                             aat_b[:, k * HN:(k + 1) * HN],
                                             start=False, stop=(k == HSTOP))
                        for j in range(2):
                            nc.tensor.matmul(
                                accs[t][j][:], lhs(t, k),
                                wt_b[:, k * O + hh * HO + j * 512:
                                     k * O + hh * HO + (j + 1) * 512],
                                start=(k == 0), stop=False)
                    if hh == 0 and k == HSTOP:
                        gts[0] = make_gt(0, hps[0])
                        gts[1] = make_gt(1, hps[1])
                for t in range(2):
                    for j in range(2):
                        nc.tensor.matmul(accs[t][j][:], gts[t][:],
                                         bt_b[:, hh * HO + j * 512:
                                              hh * HO + (j + 1) * 512],
                                         start=False, stop=True)
                for t in range(2):
                    for j in range(2):
                        drain(t, accs[t][j], 2 * hh + j)
                for t in range(2):
                    store(t, hh)

            load_x(2)
            load_x(3)

            # =========== main loop: tiles 2..15, one tile at a time ========
            for t in range(2, NT):
                out_tiles[t] = obuf.tile([P, O], bf16, tag="obuf", name=f"out_{t}")
                accs = [psA.tile([P, 512], f32, tag="acc", name=f"acc_{t}_{j}")
                        for j in range(4)]
                h = psH.tile([P, HN], f32, tag="h", name=f"h_{t}")
                h_preblock(t, h)
                gt = None
                for k in range(KT):
                    if k <= HSTOP:
                        nc.tensor.matmul(h[:], lhs(t, k),
                                         aat_b[:, k * HN:(k + 1) * HN],
                                         start=False, stop=(k == HSTOP))
                    for j in range(4):
                        nc.tensor.matmul(
                            accs[j][:], lhs(t, k),
                            wt_b[:, k * O + j * 512:k * O + (j + 1) * 512],
                            start=(k == 0), stop=False)
                    if k == HSTOP:
                        gt = make_gt(t, h)
                    if k == 8 and t + 2 < NT:
                        load_x(t + 2)
                for j in range(4):
                    nc.tensor.matmul(accs[j][:], gt[:],
                                     bt_b[:, j * 512:(j + 1) * 512],
                                     start=False, stop=True)
                if t == NT - 1:
                    # final tile: per-quarter drains + stores on 3 queues so
                    # the end-of-kernel flush backlog is small.
                    for j, q in zip(range(4),
                                    (nc.gpsimd, nc.sync, nc.scalar, nc.sync)):
                        drain(t, accs[j], j)
                        q.dma_start(
                            out_d[t * P:(t + 1) * P, j * 512:(j + 1) * 512],
                            out_tiles[t][:, j * 512:(j + 1) * 512])
                else:
                    drain(t, accs[0], 0)
                    drain(t, accs[1], 1)
                    store(t, 0)
                    drain(t, accs[2], 2)
                    drain(t, accs[3], 3)
                    store(t, 1)

    nc.compile()
    return nc


def _prep_host(x, W_base, b_base, A, B, W_router, b_router):
    """Host-side layout prep + sharding. Returns per-core input maps."""
    import ml_dtypes
    bf16 = ml_dtypes.bfloat16

    A = np.asarray(A, dtype=np.float32)
    B = np.asarray(B, dtype=np.float32)
    wr = np.asarray(W_router, dtype=np.float32)

    x_flat = np.ascontiguousarray(x, dtype=np.float32).reshape(-1, D)
    # xt[t*P + p, k*P + j] = x[t*P + j, k*P + p], per core
    NTOT = x_flat.shape[0] // P
    xt_all = np.ascontiguousarray(
        x_flat.reshape(NTOT, P, KT, P).transpose(0, 3, 2, 1)
    ).reshape(NTOT * P, D).astype(bf16)

    # W_eff = W_base + scale*B1@A1, folded on host
    w_eff = np.asarray(W_base, dtype=np.float32) + SCALE * (B[1] @ A[1])
    wt = w_eff.T                                                    # [D, O]
    wt_p = np.ascontiguousarray(
        wt.reshape(KT, P, O).transpose(1, 0, 2).reshape(P, KT * O)
    ).astype(bf16)

    a_cat = A.reshape(ER, D)                                        # [16, D]
    aat = np.zeros((D, HN), dtype=np.float32)
    aat[:, :ER] = a_cat.T
    aat[:, ER] = wr[:, 0] - wr[:, 1]
    aat_p = np.ascontiguousarray(
        aat.reshape(KT, P, HN).transpose(1, 0, 2).reshape(P, KT * HN)
    ).astype(bf16)

    b_d = np.concatenate([B[0], -B[1]], axis=1)                     # [O, 16]
    bt = np.zeros((P, O), dtype=np.float32)                         # [128, O]
    bt[:ER] = b_d.T * SCALE
    bt = np.ascontiguousarray(bt).astype(bf16)
    dlb = np.float32(b_router[0]) - np.float32(b_router[1])
    brd = np.array([[dlb]], dtype=np.float32)

    in_maps = []
    for c in range(NCORES):
        in_maps.append({
            "xt": xt_all[c * TOK:(c + 1) * TOK],
            "wt": wt_p,
            "aat": aat_p,
            "bt": bt,
            "brd": brd,
        })
    return in_maps


def kernel(x, W_base, b_base, A, B, W_router, b_router):
    from concourse import bass_utils

    if "nc" not in _CACHE:
        _CACHE["nc"] = _build()
    nc = _CACHE["nc"]

    in_maps = _prep_host(x, W_base, b_base, A, B, W_router, b_router)
    res = None
    for attempt in range(3):
        try:
            res = bass_utils.run_bass_kernel_spmd(
                nc, in_maps, core_ids=list(range(NCORES)))
            break
        except Exception:
            # rare transient NRT_EXEC_UNIT_UNRECOVERABLE observed once;
            # the same NEFF runs fine on retry
            if attempt == 2:
                raise
    out = np.concatenate([res.results[c]["out"] for c in range(NCORES)], axis=0)
    out = out.astype(np.float32) + np.asarray(b_base, dtype=np.float32)
    return out.reshape(np.asarray(x).shape[0], -1, O)
